# revision 8
# baseline (speedup 1.0000x reference)
# Trainium2 Bass kernel for nn_Attention_5102421148295.
#
# Reference computation (per batch b, X = x[b] of shape (N=4096, C=512)):
#   qkv = X @ w_qkv ; q,k,v heads of 64; sim_h = scale * q_h^T k_h (64x64)
#   attn_h = softmax_rows(sim_h); out_h = v_h attn_h^T; y = out @ w_out + b
#
# Key restructure (contraction in sim is over ALL spatial positions):
#   G    = X^T X                      (512x512, the only big LHS-pass matmul)
#   T1   = G @ Wk                     (512x512)
#   sim_h = scale * Wq_h^T @ T1_h     (64x64 per head)
#   attn_h = softmax(sim_h)
#   M_h  = attn_h^T @ w_out_h         (64x512); M = stack_h M_h (512x512)
#   P    = Wv @ M                     (512x512)
#   y    = X @ P + b_out              (4096x512, the second big pass)
# This does ~2.2x fewer FLOPs than the direct algorithm and needs no
# attention over N at all.
#
# Distribution: pure data-parallel over batch: 32 batches -> 4 per core on
# 8 cores, weights replicated, no collectives.
#
# Matmul dtype: float32r (TF32-like fast fp32; full PE rate for moving dim
# >= 256 vs 1/4 rate for plain fp32). The BIR verifier requires every
# producer of an f32r matmul input to emit dtype float32r, so those SBUF
# tiles are allocated as F32R and fp32 sources are bitcast (pure relabel;
# the PE truncates mantissas internally).

import numpy as np
from contextlib import ExitStack

import concourse.bass as bass
from concourse import bacc
import concourse.mybir as mybir
import concourse.tile as tile
from concourse.bass_utils import run_bass_kernel_spmd

F32 = mybir.dt.float32
F32R = mybir.dt.float32r

B, HH, WW, C = 32, 64, 64, 512
N = HH * WW          # 4096 spatial positions
HEADS, DH = 8, 64
SCALE = DH ** -0.5   # 0.125
N_CORES = 8
BPC = B // N_CORES   # batches per core
NT = N // 128        # spatial tiles of 128 positions
CK = C // 128        # 4 channel chunks

USE_F32R = True


def build_bass():
    MDT = F32R if USE_F32R else F32

    def rb(ap):
        # relabel an fp32 AP as the matmul dtype (same bytes)
        return ap.bitcast(F32R) if USE_F32R else ap

    nc = bacc.Bacc()
    x_in = nc.dram_tensor("x", [BPC, N, C], F32, kind="ExternalInput")
    wqkv_in = nc.dram_tensor("w_qkv", [C, 3 * C], F32, kind="ExternalInput")
    wout_in = nc.dram_tensor("w_out", [C, C], F32, kind="ExternalInput")
    bout_in = nc.dram_tensor("b_out", [C], F32, kind="ExternalInput")
    y_out = nc.dram_tensor("y", [BPC, N, C], F32, kind="ExternalOutput")

    with tile.TileContext(nc) as tc, ExitStack() as ctx:
        const = ctx.enter_context(tc.tile_pool(name="const", bufs=1))
        xtp = ctx.enter_context(tc.tile_pool(name="xt", bufs=1))
        xload = ctx.enter_context(tc.tile_pool(name="xload", bufs=4))
        midsb = ctx.enter_context(tc.tile_pool(name="midsb", bufs=1))
        soft = ctx.enter_context(tc.tile_pool(name="soft", bufs=4))
        youtp = ctx.enter_context(tc.tile_pool(name="yout", bufs=4))

        # ---------------- constants ----------------
        # identity for PE transposes: Const DRAM -> SBUF as f32r so the
        # f32r transpose matmuls see an f32r-producing DMA (verifier rule)
        ident = const.tile([128, 128], MDT)
        ident_dram = nc.inline_tensor(np.eye(128, dtype=np.float32), name="ident")
        nc.sync.dma_start(out=ident[:], in_=rb(ident_dram[:]))

        wqkv_sb = const.tile([128, CK, 3 * C], MDT)  # [p, ck, f] = w_qkv[ck*128+p, f]
        for ck in range(CK):
            nc.sync.dma_start(
                out=wqkv_sb[:, ck, :], in_=rb(wqkv_in[ck * 128:(ck + 1) * 128, :])
            )
        wout_sb = const.tile([64, HEADS, C], F32)    # [p, h, c] = w_out[h*64+p, c]
        for h in range(HEADS):
            nc.sync.dma_start(
                out=wout_sb[:, h, :], in_=wout_in[h * 64:(h + 1) * 64, :]
            )
        bias_sb = const.tile([128, C], F32)
        bout_ap = bout_in[:]
        bias_bcast = bass.AP(
            tensor=bout_ap.tensor, offset=bout_ap.offset, ap=[[0, 128], *bout_ap.ap]
        )
        nc.sync.dma_start(out=bias_sb, in_=bias_bcast)

        # WvT[f, c'] = Wv[c', f] = w_qkv[c', 2C + f]; [p, h, c'] = WvT[h*64+p, c']
        # (64-partition head chunks, all at base 0, so no PE array tiling)
        wvt_sb = const.tile([64, HEADS, C], MDT)
        with tc.tile_pool(name="wvt_ps", bufs=2, space="PSUM") as wvtps:
            for h in range(HEADS):
                for ck in range(CK):
                    pt = wvtps.tile([64, 128], MDT, tag="wvt", name=f"wvt_{h}_{ck}")
                    nc.tensor.transpose(
                        pt[:],
                        wqkv_sb[:, ck, 2 * C + h * 64: 2 * C + (h + 1) * 64],
                        ident[:],
                    )
                    nc.vector.tensor_copy(
                        out=wvt_sb[:, h, ck * 128:(ck + 1) * 128], in_=pt[:]
                    )

        for b in range(BPC):
            # ------------- phase 1: G = X^T X, and build xT -------------
            xT = xtp.tile([128, CK, N], MDT, tag="xT")  # [p, ck, d] = x[d, ck*128+p]
            G_sb = midsb.tile([128, CK, C], MDT, tag="G")
            with (
                tc.tile_pool(name="g_ps", bufs=1, space="PSUM") as gps,
                tc.tile_pool(name="t_ps", bufs=2, space="PSUM") as tps,
            ):
                g_psum = [
                    gps.tile([128, C], F32, tag=f"g{ck}", name=f"g{ck}_{b}")
                    for ck in range(CK)
                ]
                for t in range(NT):
                    x_t = xload.tile([128, C], MDT, tag="x")
                    nc.sync.dma_start(
                        out=x_t[:], in_=rb(x_in[b, t * 128:(t + 1) * 128, :])
                    )
                    for ck in range(CK):
                        nc.tensor.matmul(
                            g_psum[ck][:],
                            lhsT=x_t[:, ck * 128:(ck + 1) * 128],
                            rhs=x_t[:],
                            start=(t == 0),
                            stop=(t == NT - 1),
                        )
                    pt = tps.tile([128, C], MDT, tag="tp")
                    for ck in range(CK):
                        nc.tensor.transpose(
                            pt[:, ck * 128:(ck + 1) * 128],
                            x_t[:, ck * 128:(ck + 1) * 128],
                            ident[:],
                        )
                    nc.vector.tensor_copy(
                        out=xT[:, :, t * 128:(t + 1) * 128],
                        in_=pt[:].rearrange("p (ck d) -> p ck d", ck=CK),
                    )
                for ck in range(CK):
                    nc.vector.tensor_copy(out=G_sb[:, ck, :], in_=rb(g_psum[ck][:]))

            # ------------- phase 2: T1, sim, softmax, M, P -------------
            T1_sb = midsb.tile([128, CK, C], MDT, tag="T1")
            M_sb = midsb.tile([64, HEADS, C], MDT, tag="M")
            P_sb = midsb.tile([128, CK, C], MDT, tag="P")
            with tc.tile_pool(name="mid_ps", bufs=2, space="PSUM") as mps:
                # T1 = G @ Wk  (uses G symmetry: pass G chunks as lhsT)
                for cc in range(CK):
                    t1p = mps.tile([128, C], F32, tag="midp", name=f"t1p{cc}_{b}")
                    for ckr in range(CK):
                        nc.tensor.matmul(
                            t1p[:],
                            lhsT=G_sb[:, ckr, cc * 128:(cc + 1) * 128],
                            rhs=wqkv_sb[:, ckr, C:2 * C],
                            start=(ckr == 0),
                            stop=(ckr == CK - 1),
                        )
                    nc.vector.tensor_copy(out=T1_sb[:, cc, :], in_=rb(t1p[:]))

                # sim_h = Wq_h^T @ T1_h; head h at free columns h*64..+64 of a
                # (64, 512) psum tile, partition base 0 everywhere. Plain f32
                # matmul (inputs hold full-precision bits; n=64 is the same
                # speed either way).
                simp = mps.tile([64, HEADS * DH], F32, tag="simp")
                for h in range(HEADS):
                    for ck in range(CK):
                        nc.tensor.matmul(
                            simp[:, h * 64:(h + 1) * 64],
                            lhsT=wqkv_sb[:, ck, h * 64:(h + 1) * 64].bitcast(F32),
                            rhs=T1_sb[:, ck, h * 64:(h + 1) * 64].bitcast(F32),
                            start=(ck == 0),
                            stop=(ck == CK - 1),
                        )

                # softmax (1/8 scale folded into Exp) + M_h = attn_h^T w_out_h
                for h in range(HEADS):
                    hsim = simp[:, h * 64:(h + 1) * 64]
                    mx = soft.tile([64, 1], F32, tag="mx")
                    nc.vector.reduce_max(
                        out=mx[:], in_=hsim, axis=mybir.AxisListType.X
                    )
                    nm = soft.tile([64, 1], F32, tag="nm")
                    nc.scalar.mul(nm[:], mx[:], -SCALE)
                    at = soft.tile([64, DH], F32, tag="at")
                    ssum = soft.tile([64, 1], F32, tag="ssum")
                    nc.scalar.activation(
                        out=at[:],
                        in_=hsim,
                        func=mybir.ActivationFunctionType.Exp,
                        bias=nm[:],
                        scale=SCALE,
                        accum_out=ssum[:],
                    )
                    rinv = soft.tile([64, 1], F32, tag="rinv")
                    nc.vector.reciprocal(rinv[:], ssum[:])
                    nc.vector.tensor_scalar_mul(at[:], at[:], rinv[:])
                    mp8 = mps.tile([64, C], F32, tag="midp8", name=f"mp{h}_{b}")
                    nc.tensor.matmul(
                        mp8[:], lhsT=at[:], rhs=wout_sb[:, h, :],
                        start=True, stop=True,
                    )
                    nc.vector.tensor_copy(out=M_sb[:, h, :], in_=rb(mp8[:]))

                # P = Wv @ M  (via WvT head chunks as lhsT, K=64 per chunk)
                for cp in range(CK):
                    pp = mps.tile([128, C], F32, tag="midp", name=f"pp{cp}_{b}")
                    for h in range(HEADS):
                        nc.tensor.matmul(
                            pp[:],
                            lhsT=wvt_sb[:, h, cp * 128:(cp + 1) * 128],
                            rhs=M_sb[:, h, :],
                            start=(h == 0),
                            stop=(h == HEADS - 1),
                        )
                    nc.vector.tensor_copy(out=P_sb[:, cp, :], in_=rb(pp[:]))

            # ------------- phase 3: y = X @ P + b -------------
            with tc.tile_pool(name="y_ps", bufs=3, space="PSUM") as yps:
                for dk in range(NT):
                    yp = yps.tile([128, C], F32, tag="yp")
                    for ck in range(CK):
                        nc.tensor.matmul(
                            yp[:],
                            lhsT=xT[:, ck, dk * 128:(dk + 1) * 128],
                            rhs=P_sb[:, ck, :],
                            start=(ck == 0),
                            stop=(ck == CK - 1),
                        )
                    y_sb = youtp.tile([128, C], F32, tag="ysb")
                    nc.vector.tensor_add(y_sb[:], yp[:], bias_sb[:])
                    nc.sync.dma_start(
                        out=y_out[b, dk * 128:(dk + 1) * 128, :], in_=y_sb[:]
                    )

    nc.finalize()
    return nc


_NC_CACHE = None


def _get_nc():
    global _NC_CACHE
    if _NC_CACHE is None:
        _NC_CACHE = build_bass()
    return _NC_CACHE


def _make_in_maps(x, w_qkv, w_out, b_out):
    x = np.ascontiguousarray(np.asarray(x, dtype=np.float32)).reshape(B, N, C)
    w_qkv = np.ascontiguousarray(np.asarray(w_qkv, dtype=np.float32))
    w_out = np.ascontiguousarray(np.asarray(w_out, dtype=np.float32))
    b_out = np.ascontiguousarray(np.asarray(b_out, dtype=np.float32))
    return [
        {
            "x": np.ascontiguousarray(x[c * BPC:(c + 1) * BPC]),
            "w_qkv": w_qkv,
            "w_out": w_out,
            "b_out": b_out,
        }
        for c in range(N_CORES)
    ]


def run(x, w_qkv, w_out, b_out, trace=False, **kw):
    """Run on 8 cores; returns (full y (B,H,W,C), BassKernelResults)."""
    in_maps = _make_in_maps(x, w_qkv, w_out, b_out)
    res = run_bass_kernel_spmd(
        _get_nc(), in_maps, core_ids=list(range(N_CORES)), trace=trace, **kw
    )
    y = np.concatenate([r["y"] for r in res.results], axis=0)
    return y.reshape(B, HH, WW, C).astype(np.float32), res


def kernel(x, w_qkv, w_out, b_out):
    y, _ = run(x, w_qkv, w_out, b_out)
    return y
